# revision 10
# baseline (speedup 1.0000x reference)
"""CloudRasterizerOversample Trainium2 kernel (v3).

Strategy
--------
Splat + 4x4x4 mean-pool is linear, so the pooled 64x128x128 cube is
built directly: the weight of a point to a lo-res cell along one axis
is a trapezoid t(u) = relu(min(u, 5-u, 1)) (u = g - 4c + 1) with
support on at most 2 consecutive cells.

Sharding: core k owns v-planes 8k..8k+7.  A point contributes to <=2
v-planes (p, p+1); one entry carries BOTH plane weights (tv0, tv1) and
the matmul writes both plane strips of PSUM in a single instruction
via a 2-block strided output access pattern (plane 7 spills into a
dumpster strip; the p+1 contribution is re-routed to the next core as
a duplicate entry).

Within a core, entries are binned by (plane, y-block of 16 cells,
x-block of 8).  Because a 16-wide weight tile must sit at a 32-aligned
PE column, PSUM holds TWO images (even/odd y-block parity); y-block
2r+par lands at partition base 32r of image par.  For each 256-column
group the DVE builds, with one fused trapezoid instruction per tensor,
    AY[128, 256, 16] = fw * t_y      (fw = flux/64)
    AX[128, 256, 2, 9]: slot s = tv_s * t_x
and per column one PE matmul accumulates
    img_par[32r:+16, (pl:pl+2)*128 + xblk*8 +: 9] += AY_c^T @ AX_c
All host-side address math (grid coords, per-column scan offsets) is
precomputed into the DMA'd arrays, so the device only runs TRAP + MM.
"""

import os
import sys
import numpy as np
from contextlib import ExitStack

import concourse.bass as bass
import concourse.bacc as bacc
import concourse.mybir as mybir
import concourse.tile as tile
from concourse.bass_utils import run_bass_kernel_spmd

# ---------------- problem constants (hardcoded per spec) ----------------
N_PIX_LO = 128
OV_XY = 4
OV_V = 4
NV_LO = 64
PIX_LO = 0.1
VEL0_LO = -400.0
DV_LO = 12.5
N_PIX_HI = N_PIX_LO * OV_XY            # 512
PIX_HI = PIX_LO / OV_XY                # 0.025
FOV_HALF_HI = 0.5 * (N_PIX_HI - 1) * PIX_HI
DV_HI = DV_LO / OV_V                   # 3.125
VEL0_HI = VEL0_LO - 0.5 * (DV_LO - DV_HI)
NV_HI = NV_LO * OV_V                   # 256

N_CORES = 8
PLANES = NV_LO // N_CORES              # 8 base v-planes per core
NYB = 8                                # y blocks of 16 cells
NXB = 16                               # x blocks of 8 cells
WY = 16                                # y window width
WX = 9                                 # x window width
GRP = 128                              # columns per compute group
DMAG = int(os.environ.get('KDMAG', '128'))  # columns per DMA slice
AYSEG = 64                             # columns per ay DVE call (1024 elems)
AXSEG = 128                            # columns per ax DVE call (1152 elems)
CHUNK = 128
NBINS = PLANES * NYB * NXB             # 1024 bins per core

# device scalars (f32)
INV_P = float(np.float32(1.0 / PIX_HI))
OFF_P = float(np.float32(FOV_HALF_HI / PIX_HI))
INV_DV = float(np.float32(1.0 / DV_HI))
VOFF = float(np.float32(-VEL0_HI / DV_HI))

_DBG = os.environ.get("KERNEL_DEBUG", "") != ""


def _log(*a):
    if _DBG:
        print("[kernel]", *a, file=sys.stderr, flush=True)


# ---------------- custom DVE op ----------------
from concourse.dve_spec import (
    Spec, Src0, Src1, C0, C1, Zero, One, AluOp, Bin, relu, minn, lower, scan,
)
from concourse.dve_ops import DveOp, OPS, CUSTOM_DVE_SPECS, _SUB_OPCODE_FOR_NAME
from concourse.dve_uop import DveOpSpec


def _trap_ref(in0, in1, c0, c1, c2):
    """out = in0 * relu(min(min(v, (1-v)+4), 1)), v = in1 - 4*Idx (global)."""
    in0 = np.asarray(in0, np.float32)
    in1 = np.asarray(in1, np.float32)
    n = int(np.prod(in0.shape[1:]))
    scan4 = (np.arange(n, dtype=np.float32) * np.float32(4.0)).reshape(in0.shape[1:])
    v = (in1 - scan4[None]).astype(np.float32)
    b = ((np.float32(1.0) - v) + np.float32(4.0)).astype(np.float32)
    m = np.minimum(np.minimum(v, b), np.float32(1.0))
    r = np.maximum(m, np.float32(0.0))
    return (in0 * r).astype(np.float32)


_scan4 = scan(AluOp.ADD, C1, init=Bin(AluOp.SUBTRACT, Zero, C1))
_v = Src1 - _scan4
TRAP_SPEC = Spec(body=Src0 * relu(minn(minn(_v, (One - _v) + C1), One)),
                 reference=_trap_ref)


def _mk_op(name, spec):
    if name in _SUB_OPCODE_FOR_NAME:
        for op in OPS:
            if op.name == name:
                return op
    shas = {}
    for ver in ("v3", "v4"):
        uops = lower(spec, ver=ver)
        row = max(_SUB_OPCODE_FOR_NAME.values()) + 1
        shas[ver] = DveOpSpec(name=name, opcode=row, uops=uops, rd1_en=True).sha(ver)
    op = DveOp(name, spec, subdim=False, uops_sha=shas)
    OPS.append(op)
    _SUB_OPCODE_FOR_NAME[name] = max(_SUB_OPCODE_FOR_NAME.values()) + 1
    CUSTOM_DVE_SPECS[name] = spec
    return op


TRAP_OP = _mk_op("RAST_TRAP_ANT", TRAP_SPEC)


# ---------------- host-side routing ----------------
def route_points(ra, dec, vel, flux):
    """Shard points by base v-plane across cores; bin by (plane, yblk, xblk).

    Returns (per_core [list of dict name->np array], consts dict (empty),
    chunk_tbl [C,3] int array of (plane, yblk, xblk), C).
    """
    f32 = np.float32
    ra = np.asarray(ra, f32)
    dec = np.asarray(dec, f32)
    vel = np.asarray(vel, f32)
    flux = np.asarray(flux, f32)

    # validity, exactly as the reference computes it (f32 add, f32 divide)
    def ref_idx(arr, off, scale):
        q = ((arr + f32(off)) / f32(scale)).astype(f32)
        return np.floor(q).astype(np.int64)

    ix0 = ref_idx(ra, FOV_HALF_HI, PIX_HI)
    iy0 = ref_idx(dec, FOV_HALF_HI, PIX_HI)
    iv0 = ref_idx(vel, -VEL0_HI, DV_HI)
    valid = ((ix0 >= 0) & (ix0 < N_PIX_HI - 1) &
             (iy0 >= 0) & (iy0 < N_PIX_HI - 1) &
             (iv0 >= 0) & (iv0 < NV_HI - 1))

    ra_v = ra[valid]
    dec_v = dec[valid]
    vel_v = vel[valid]
    flux_v = flux[valid]

    # device-order grid coords (f32 mult + add); f64 copies for exact floors
    gx32 = (ra_v * f32(INV_P) + f32(OFF_P)).astype(f32)
    gy32 = (dec_v * f32(INV_P) + f32(OFF_P)).astype(f32)
    gv32 = (vel_v * f32(INV_DV) + f32(VOFF)).astype(f32)
    gxd = gx32.astype(np.float64)
    gyd = gy32.astype(np.float64)
    gvd = gv32.astype(np.float64)

    cx = (np.floor((gxd - 4.0) / 4.0) + 1).astype(np.int64)
    cy = (np.floor((gyd - 4.0) / 4.0) + 1).astype(np.int64)
    cv = (np.floor((gvd - 4.0) / 4.0) + 1).astype(np.int64)
    np.clip(cx, 0, N_PIX_LO - 1, out=cx)
    np.clip(cy, 0, N_PIX_LO - 1, out=cy)
    np.clip(cv, 0, NV_LO - 1, out=cv)

    def trapv(u):
        m = np.minimum(np.minimum(u + f32(1.0), f32(4.0) - u), f32(1.0))
        return np.maximum(m, f32(0.0)).astype(f32)

    tv0 = trapv(gv32 - (4.0 * cv).astype(f32))
    tv1 = trapv(gv32 - (4.0 * (cv + 1)).astype(f32))

    n = ra_v.shape[0]
    pid = np.arange(n)

    # v core-boundary duplication: base plane local 7 with spill into the
    # next core's plane 0
    bdup = ((cv & 7) == 7) & (tv1 > 0)
    e_pid = np.concatenate([pid, pid[bdup]])
    e_pl = np.concatenate([cv, cv[bdup] + 1])       # global base plane
    e_tv0 = np.concatenate([tv0, tv1[bdup]])
    e_tv1 = np.concatenate([tv1, np.zeros(int(bdup.sum()), f32)])

    # y duplication at y-block boundary
    e_cy = cy[e_pid]
    sy = gyd[e_pid] > 4.0 * e_cy + 3.0              # t(cy+1) > 0
    ydup = sy & ((e_cy & (WY - 1)) == WY - 1) & (e_cy < N_PIX_LO - 1)
    f_pid = np.concatenate([e_pid, e_pid[ydup]])
    f_pl = np.concatenate([e_pl, e_pl[ydup]])
    f_tv0 = np.concatenate([e_tv0, e_tv0[ydup]])
    f_tv1 = np.concatenate([e_tv1, e_tv1[ydup]])
    f_yb = np.concatenate([e_cy >> 4, (e_cy[ydup] >> 4) + 1])

    f_xblk = cx[f_pid] >> 3
    f_core = f_pl >> 3
    f_plane = f_pl & 7
    f_bin = (f_plane * NYB + f_yb) * NXB + f_xblk
    f_str = f_tv1 > 0                               # needs the 2nd v-plane slot

    key = f_core * NBINS + f_bin
    counts = np.bincount(key, minlength=N_CORES * NBINS).reshape(N_CORES, NBINS)
    scount = np.bincount(key[f_str], minlength=N_CORES * NBINS).reshape(
        N_CORES, NBINS)
    maxc = counts.max(axis=0)
    nchunks = (maxc + CHUNK - 1) // CHUNK           # 0 for empty bins
    n2 = (scount.max(axis=0) + CHUNK - 1) // CHUNK  # 2-slot chunks per bin
    n1 = nchunks - n2

    # chunk table (shared across cores); 2-slot columns first globally
    plane_b, rem = np.divmod(np.arange(NBINS), NYB * NXB)
    yb_b, xblk_b = np.divmod(rem, NXB)
    chunk_plane = np.concatenate([np.repeat(plane_b, n2), np.repeat(plane_b, n1)])
    chunk_yb = np.concatenate([np.repeat(yb_b, n2), np.repeat(yb_b, n1)])
    chunk_xblk = np.concatenate([np.repeat(xblk_b, n2), np.repeat(xblk_b, n1)])
    C2 = int(n2.sum())
    C = chunk_plane.shape[0]
    chunk_tbl = np.stack([chunk_plane, chunk_yb, chunk_xblk], axis=1)

    start2 = np.zeros(NBINS, np.int64)
    np.cumsum(n2[:-1], out=start2[1:])
    start1 = np.zeros(NBINS, np.int64)
    np.cumsum(n1[:-1], out=start1[1:])
    start1 += C2

    # straddlers first within each (core, bin) group
    order = np.argsort(key * 2 + (1 - f_str.astype(np.int64)), kind="stable")
    key_s = key[order]
    group_start = np.searchsorted(key_s, key_s)     # first occurrence index
    rank = np.arange(key_s.shape[0]) - group_start
    j = rank // CHUNK
    bo = f_bin[order]
    col = np.where(j < n2[bo], start2[bo] + j, start1[bo] + (j - n2[bo]))
    lane = rank % CHUNK
    core_s = f_core[order]

    # per-entry precomputed device values (offsets use the entry's column)
    colmod_x = (col % AXSEG).astype(np.float64)
    colmod_y = (col % AYSEG).astype(np.float64)
    gx_e = (gx32[f_pid[order]]
            + (1.0 - 32.0 * f_xblk[order] + 36.0 * colmod_x).astype(f32)).astype(f32)
    gy_e = (gy32[f_pid[order]]
            + (1.0 - 4.0 * WY * f_yb[order] + 4.0 * WY * colmod_y).astype(f32)
            ).astype(f32)
    fw_e = (flux_v[f_pid[order]] / f32(64.0)).astype(np.float16)
    tv0_e = f_tv0[order].astype(np.float16)
    tv1_e = f_tv1[order].astype(np.float16)

    # per-column pad base values (benign: fw/tv pads are zero)
    colidx = np.arange(C)
    base_gx = (1.0 - 32.0 * chunk_xblk + 36.0 * (colidx % AXSEG)).astype(f32)
    base_gy = (1.0 - 4.0 * WY * chunk_yb + 4.0 * WY * (colidx % AYSEG)).astype(f32)

    per_core = []
    for k in range(N_CORES):
        m = core_s == k
        cols_k = col[m]
        lanes_k = lane[m]

        a_gx = np.empty((C, CHUNK), f32)
        a_gy = np.empty((C, CHUNK), f32)
        a_gx[:] = base_gx[:, None]
        a_gy[:] = base_gy[:, None]
        a_fw = np.zeros((C, CHUNK), np.float16)
        a_tv0 = np.zeros((C, CHUNK), np.float16)
        a_tv1 = np.zeros((C, CHUNK), np.float16)
        a_gx[cols_k, lanes_k] = gx_e[m]
        a_gy[cols_k, lanes_k] = gy_e[m]
        a_fw[cols_k, lanes_k] = fw_e[m]
        a_tv0[cols_k, lanes_k] = tv0_e[m]
        a_tv1[cols_k, lanes_k] = tv1_e[m]

        per_core.append({
            "gx": np.ascontiguousarray(a_gx.T),
            "gy": np.ascontiguousarray(a_gy.T),
            "fw": np.ascontiguousarray(a_fw.T),
            "tv0": np.ascontiguousarray(a_tv0.T),
            "tv1": np.ascontiguousarray(a_tv1.T),
        })

    return per_core, {"n_real_cols": C, "c2": C2}, chunk_tbl, C


# ---------------- device kernel ----------------
def build_kernel(C, chunk_tbl, num_devices=N_CORES, mm_bf16=True, n_real_cols=None,
                 c2=None):
    f = mybir.dt.float32
    h = mybir.dt.float16
    bf = mybir.dt.bfloat16
    if n_real_cols is None:
        n_real_cols = C
    if c2 is None:
        c2 = C
    nc = bacc.Bacc("TRN2", target_bir_lowering=False, debug=False,
                   enable_asserts=False, num_devices=num_devices)
    d_in = {}
    for nm, dt_ in (("gx", f), ("gy", f), ("fw", h), ("tv0", h), ("tv1", h)):
        d_in[nm] = nc.dram_tensor(nm, [CHUNK, C], dt_, kind="ExternalInput")
    d_out = [nc.dram_tensor(f"out{p}", [CHUNK, PLANES * N_PIX_LO], f,
                            kind="ExternalOutput") for p in range(2)]

    with tile.TileContext(nc) as tc, ExitStack() as ctx:
        pool = ctx.enter_context(tc.tile_pool(name="sbuf", bufs=1))
        aypool = ctx.enter_context(tc.tile_pool(name="ay", bufs=3))
        axpool = ctx.enter_context(tc.tile_pool(name="ax", bufs=3))
        ppool = ctx.enter_context(tc.tile_pool(name="psum", bufs=1, space="PSUM"))

        t = {}
        for nm, dt_ in (("gx", f), ("gy", f), ("fw", h), ("tv0", h), ("tv1", h)):
            t[nm] = pool.tile([CHUNK, C], dt_, tag=nm, name=f"t_{nm}")

        zl = pool.tile([CHUNK, CHUNK], bf, tag="zl")
        zr = pool.tile([CHUNK, 512], bf, tag="zr")
        nc.vector.memset(zl[:], 0.0)
        nc.vector.memset(zr[:], 0.0)

        # two psum images (y-block parity); 8 plane strips + 1 dumpster each
        imgs = [ppool.tile([CHUNK, PLANES + 1, N_PIX_LO], f, tag=f"img{p}",
                           space="PSUM", name=f"img{p}") for p in range(2)]
        for img in imgs:
            nc.tensor.matmul(out=img[:, 0:4, :], lhsT=zl[:], rhs=zr[:],
                             start=True, stop=False)
            nc.tensor.matmul(out=img[:, 4:8, :], lhsT=zl[:], rhs=zr[:],
                             start=True, stop=False)
            nc.tensor.matmul(out=img[:, 8:9, :], lhsT=zl[:], rhs=zr[:, 0:128],
                             start=True, stop=False)

        dma_done = 0
        for g0 in range(0, C, GRP):
            gn = min(GRP, C - g0)
            while dma_done < g0 + gn:               # prefetch in DMAG slices
                dn = min(DMAG, C - dma_done)
                dsl = slice(dma_done, dma_done + dn)
                for nm in ("gx", "gy", "fw", "tv0", "tv1"):
                    nc.sync.dma_start(out=t[nm][:, dsl], in_=d_in[nm].ap()[:, dsl])
                dma_done += dn
            n2g = max(0, min(gn, c2 - g0))          # 2-slot cols in this group

            ay = aypool.tile([CHUNK, GRP, WY], bf, tag="ay")
            for a0 in range(0, gn, AYSEG):
                an = min(AYSEG, gn - a0)
                asl = slice(g0 + a0, g0 + a0 + an)
                nc.vector._custom_dve(
                    TRAP_OP, out=ay[:, a0:a0 + an, :],
                    in0=t["fw"][:, asl, None].to_broadcast([CHUNK, an, WY]),
                    in1=t["gy"][:, asl, None].to_broadcast([CHUNK, an, WY]),
                    s1=4.0)
            axp = axpool.tile([CHUNK, GRP, 2, WX], bf, tag="axp")
            for a0 in range(0, gn, AXSEG):
                an = min(AXSEG, gn - a0)
                asl = slice(g0 + a0, g0 + a0 + an)
                nc.vector._custom_dve(
                    TRAP_OP, out=axp[:, a0:a0 + an, 0, :],
                    in0=t["tv0"][:, asl, None].to_broadcast([CHUNK, an, WX]),
                    in1=t["gx"][:, asl, None].to_broadcast([CHUNK, an, WX]),
                    s1=4.0)
            for a0 in range(0, n2g, AXSEG):
                an = min(AXSEG, n2g - a0)
                asl = slice(g0 + a0, g0 + a0 + an)
                nc.vector._custom_dve(
                    TRAP_OP, out=axp[:, a0:a0 + an, 1, :],
                    in0=t["tv1"][:, asl, None].to_broadcast([CHUNK, an, WX]),
                    in1=t["gx"][:, asl, None].to_broadcast([CHUNK, an, WX]),
                    s1=4.0)

            for c in range(g0, min(g0 + gn, n_real_cols)):
                plane, yb, xblk = (int(chunk_tbl[c, 0]),
                                   int(chunk_tbl[c, 1]),
                                   int(chunk_tbl[c, 2]))
                par = yb & 1
                r = yb >> 1
                wx = min(WX, N_PIX_LO - xblk * 8)
                nsl = 2 if c < c2 else 1
                nc.tensor.matmul(
                    out=imgs[par][32 * r:32 * r + WY, plane:plane + nsl,
                                  xblk * 8:xblk * 8 + wx],
                    lhsT=ay[:, c - g0, :],
                    rhs=axp[:, c - g0, 0:nsl, 0:wx],
                    start=False, stop=False,
                    tile_position=(0, 32 * r))

        for img in imgs:
            nc.tensor.matmul(out=img[:, 0:4, :], lhsT=zl[:], rhs=zr[:],
                             start=False, stop=True)
            nc.tensor.matmul(out=img[:, 4:8, :], lhsT=zl[:], rhs=zr[:],
                             start=False, stop=True)
            nc.tensor.matmul(out=img[:, 8:9, :], lhsT=zl[:], rhs=zr[:, 0:128],
                             start=False, stop=True)

        for p in range(2):
            ot = pool.tile([CHUNK, PLANES * N_PIX_LO], f, tag=f"ot{p}")
            nc.scalar.copy(out=ot[:], in_=imgs[p][:, 0:PLANES, :])
            nc.sync.dma_start(out=d_out[p].ap(), in_=ot[:])

    nc.compile()
    return nc


def assemble(results):
    cube = np.empty((NV_LO, N_PIX_LO, N_PIX_LO), np.float32)
    for k in range(N_CORES):
        # img_par partition 32r+m (m<16) holds y cell 32r + 16*par + m
        for par in range(2):
            res = results[k][f"out{par}"].reshape(4, 32, PLANES, N_PIX_LO)
            # res[r, m] valid only for m < 16
            sub = res[:, 0:16]                      # [4, 16, PLANES, 128]
            for r in range(4):
                y0 = 32 * r + 16 * par
                cube[k * PLANES:(k + 1) * PLANES, y0:y0 + 16] = (
                    sub[r].transpose(1, 0, 2))
    return cube


# ---------------- entry point ----------------
def kernel(ra, dec, vel, flux):
    per_core, consts, chunk_tbl, C = route_points(ra, dec, vel, flux)
    if C == 0:  # no valid points at all
        return np.zeros((NV_LO, N_PIX_LO, N_PIX_LO), np.float32)
    _log(f"C={C} columns ({C * CHUNK} entry slots)")
    nc = build_kernel(C, chunk_tbl, n_real_cols=consts["n_real_cols"],
                      c2=consts["c2"])
    in_maps = [dict(per_core[k]) for k in range(N_CORES)]
    res = run_bass_kernel_spmd(nc, in_maps, core_ids=list(range(N_CORES)))
    return assemble(res.results)


# revision 11
# speedup vs baseline: 1.1910x; 1.1910x over previous
"""CloudRasterizerOversample Trainium2 kernel (v3).

Strategy
--------
Splat + 4x4x4 mean-pool is linear, so the pooled 64x128x128 cube is
built directly: the weight of a point to a lo-res cell along one axis
is a trapezoid t(u) = relu(min(u, 5-u, 1)) (u = g - 4c + 1) with
support on at most 2 consecutive cells.

Sharding: core k owns v-planes 8k..8k+7.  A point contributes to <=2
v-planes (p, p+1); one entry carries BOTH plane weights (tv0, tv1) and
the matmul writes both plane strips of PSUM in a single instruction
via a 2-block strided output access pattern (plane 7 spills into a
dumpster strip; the p+1 contribution is re-routed to the next core as
a duplicate entry).

Within a core, entries are binned by (plane, y-block of 16 cells,
x-block of 8).  Because a 16-wide weight tile must sit at a 32-aligned
PE column, PSUM holds TWO images (even/odd y-block parity); y-block
2r+par lands at partition base 32r of image par.  For each 256-column
group the DVE builds, with one fused trapezoid instruction per tensor,
    AY[128, 256, 16] = fw * t_y      (fw = flux/64)
    AX[128, 256, 2, 9]: slot s = tv_s * t_x
and per column one PE matmul accumulates
    img_par[32r:+16, (pl:pl+2)*128 + xblk*8 +: 9] += AY_c^T @ AX_c
All host-side address math (grid coords, per-column scan offsets) is
precomputed into the DMA'd arrays, so the device only runs TRAP + MM.
"""

import os
import sys
import numpy as np
from contextlib import ExitStack

import concourse.bass as bass
import concourse.bacc as bacc
import concourse.mybir as mybir
import concourse.tile as tile
from concourse.bass_utils import run_bass_kernel_spmd

# ---------------- problem constants (hardcoded per spec) ----------------
N_PIX_LO = 128
OV_XY = 4
OV_V = 4
NV_LO = 64
PIX_LO = 0.1
VEL0_LO = -400.0
DV_LO = 12.5
N_PIX_HI = N_PIX_LO * OV_XY            # 512
PIX_HI = PIX_LO / OV_XY                # 0.025
FOV_HALF_HI = 0.5 * (N_PIX_HI - 1) * PIX_HI
DV_HI = DV_LO / OV_V                   # 3.125
VEL0_HI = VEL0_LO - 0.5 * (DV_LO - DV_HI)
NV_HI = NV_LO * OV_V                   # 256

N_CORES = 8
PLANES = NV_LO // N_CORES              # 8 base v-planes per core
NYB = 8                                # y blocks of 16 cells
NXB = 16                               # x blocks of 8 cells
WY = 16                                # y window width
WX = 9                                 # x window width
GRP = 128                              # columns per compute group
AYSEG = 64                             # columns per ay DVE call (1024 elems)
AXSEG = 128                            # columns per ax DVE call (1152 elems)
CHUNK = 128
NBINS = PLANES * NYB * NXB             # 1024 bins per core

# device scalars (f32)
INV_P = float(np.float32(1.0 / PIX_HI))
OFF_P = float(np.float32(FOV_HALF_HI / PIX_HI))
INV_DV = float(np.float32(1.0 / DV_HI))
VOFF = float(np.float32(-VEL0_HI / DV_HI))

_DBG = os.environ.get("KERNEL_DEBUG", "") != ""


def _log(*a):
    if _DBG:
        print("[kernel]", *a, file=sys.stderr, flush=True)


# ---------------- custom DVE op ----------------
from concourse.dve_spec import (
    Spec, Src0, Src1, C0, C1, Zero, One, AluOp, Bin, relu, minn, lower, scan,
)
from concourse.dve_ops import DveOp, OPS, CUSTOM_DVE_SPECS, _SUB_OPCODE_FOR_NAME
from concourse.dve_uop import DveOpSpec


def _trap_ref(in0, in1, c0, c1, c2):
    """out = in0 * relu(min(min(v, (1-v)+4), 1)), v = in1 - 4*Idx (global)."""
    in0 = np.asarray(in0, np.float32)
    in1 = np.asarray(in1, np.float32)
    n = int(np.prod(in0.shape[1:]))
    scan4 = (np.arange(n, dtype=np.float32) * np.float32(4.0)).reshape(in0.shape[1:])
    v = (in1 - scan4[None]).astype(np.float32)
    b = ((np.float32(1.0) - v) + np.float32(4.0)).astype(np.float32)
    m = np.minimum(np.minimum(v, b), np.float32(1.0))
    r = np.maximum(m, np.float32(0.0))
    return (in0 * r).astype(np.float32)


_scan4 = scan(AluOp.ADD, C1, init=Bin(AluOp.SUBTRACT, Zero, C1))
_v = Src1 - _scan4
TRAP_SPEC = Spec(body=Src0 * relu(minn(minn(_v, (One - _v) + C1), One)),
                 reference=_trap_ref)


def _mk_op(name, spec):
    if name in _SUB_OPCODE_FOR_NAME:
        for op in OPS:
            if op.name == name:
                return op
    shas = {}
    for ver in ("v3", "v4"):
        uops = lower(spec, ver=ver)
        row = max(_SUB_OPCODE_FOR_NAME.values()) + 1
        shas[ver] = DveOpSpec(name=name, opcode=row, uops=uops, rd1_en=True).sha(ver)
    op = DveOp(name, spec, subdim=False, uops_sha=shas)
    OPS.append(op)
    _SUB_OPCODE_FOR_NAME[name] = max(_SUB_OPCODE_FOR_NAME.values()) + 1
    CUSTOM_DVE_SPECS[name] = spec
    return op


TRAP_OP = _mk_op("RAST_TRAP_ANT", TRAP_SPEC)


# ---------------- host-side routing ----------------
def route_points(ra, dec, vel, flux):
    """Shard points by base v-plane across cores; bin by (plane, yblk, xblk).

    Returns (per_core [list of dict name->np array], consts dict (empty),
    chunk_tbl [C,3] int array of (plane, yblk, xblk), C).
    """
    f32 = np.float32
    ra = np.asarray(ra, f32)
    dec = np.asarray(dec, f32)
    vel = np.asarray(vel, f32)
    flux = np.asarray(flux, f32)

    # validity, exactly as the reference computes it (f32 add, f32 divide)
    def ref_idx(arr, off, scale):
        q = ((arr + f32(off)) / f32(scale)).astype(f32)
        return np.floor(q).astype(np.int64)

    ix0 = ref_idx(ra, FOV_HALF_HI, PIX_HI)
    iy0 = ref_idx(dec, FOV_HALF_HI, PIX_HI)
    iv0 = ref_idx(vel, -VEL0_HI, DV_HI)
    valid = ((ix0 >= 0) & (ix0 < N_PIX_HI - 1) &
             (iy0 >= 0) & (iy0 < N_PIX_HI - 1) &
             (iv0 >= 0) & (iv0 < NV_HI - 1))

    ra_v = ra[valid]
    dec_v = dec[valid]
    vel_v = vel[valid]
    flux_v = flux[valid]

    # device-order grid coords (f32 mult + add); f64 copies for exact floors
    gx32 = (ra_v * f32(INV_P) + f32(OFF_P)).astype(f32)
    gy32 = (dec_v * f32(INV_P) + f32(OFF_P)).astype(f32)
    gv32 = (vel_v * f32(INV_DV) + f32(VOFF)).astype(f32)
    gxd = gx32.astype(np.float64)
    gyd = gy32.astype(np.float64)
    gvd = gv32.astype(np.float64)

    cx = (np.floor((gxd - 4.0) / 4.0) + 1).astype(np.int64)
    cy = (np.floor((gyd - 4.0) / 4.0) + 1).astype(np.int64)
    cv = (np.floor((gvd - 4.0) / 4.0) + 1).astype(np.int64)
    np.clip(cx, 0, N_PIX_LO - 1, out=cx)
    np.clip(cy, 0, N_PIX_LO - 1, out=cy)
    np.clip(cv, 0, NV_LO - 1, out=cv)

    def trapv(u):
        m = np.minimum(np.minimum(u + f32(1.0), f32(4.0) - u), f32(1.0))
        return np.maximum(m, f32(0.0)).astype(f32)

    tv0 = trapv(gv32 - (4.0 * cv).astype(f32))
    tv1 = trapv(gv32 - (4.0 * (cv + 1)).astype(f32))

    n = ra_v.shape[0]
    pid = np.arange(n)

    # v core-boundary duplication: base plane local 7 with spill into the
    # next core's plane 0
    bdup = ((cv & 7) == 7) & (tv1 > 0)
    e_pid = np.concatenate([pid, pid[bdup]])
    e_pl = np.concatenate([cv, cv[bdup] + 1])       # global base plane
    e_tv0 = np.concatenate([tv0, tv1[bdup]])
    e_tv1 = np.concatenate([tv1, np.zeros(int(bdup.sum()), f32)])

    # y duplication at y-block boundary
    e_cy = cy[e_pid]
    sy = gyd[e_pid] > 4.0 * e_cy + 3.0              # t(cy+1) > 0
    ydup = sy & ((e_cy & (WY - 1)) == WY - 1) & (e_cy < N_PIX_LO - 1)
    f_pid = np.concatenate([e_pid, e_pid[ydup]])
    f_pl = np.concatenate([e_pl, e_pl[ydup]])
    f_tv0 = np.concatenate([e_tv0, e_tv0[ydup]])
    f_tv1 = np.concatenate([e_tv1, e_tv1[ydup]])
    f_yb = np.concatenate([e_cy >> 4, (e_cy[ydup] >> 4) + 1])

    f_xblk = cx[f_pid] >> 3
    f_core = f_pl >> 3
    f_plane = f_pl & 7
    f_bin = (f_plane * NYB + f_yb) * NXB + f_xblk
    f_str = f_tv1 > 0                               # needs the 2nd v-plane slot

    key = f_core * NBINS + f_bin
    counts = np.bincount(key, minlength=N_CORES * NBINS).reshape(N_CORES, NBINS)
    scount = np.bincount(key[f_str], minlength=N_CORES * NBINS).reshape(
        N_CORES, NBINS)
    maxc = counts.max(axis=0)
    nchunks = (maxc + CHUNK - 1) // CHUNK           # 0 for empty bins
    n2 = (scount.max(axis=0) + CHUNK - 1) // CHUNK  # 2-slot chunks per bin
    n1 = nchunks - n2

    # chunk table (shared across cores); 2-slot columns first globally
    plane_b, rem = np.divmod(np.arange(NBINS), NYB * NXB)
    yb_b, xblk_b = np.divmod(rem, NXB)
    chunk_plane = np.concatenate([np.repeat(plane_b, n2), np.repeat(plane_b, n1)])
    chunk_yb = np.concatenate([np.repeat(yb_b, n2), np.repeat(yb_b, n1)])
    chunk_xblk = np.concatenate([np.repeat(xblk_b, n2), np.repeat(xblk_b, n1)])
    C2 = int(n2.sum())
    C = chunk_plane.shape[0]
    chunk_tbl = np.stack([chunk_plane, chunk_yb, chunk_xblk], axis=1)

    start2 = np.zeros(NBINS, np.int64)
    np.cumsum(n2[:-1], out=start2[1:])
    start1 = np.zeros(NBINS, np.int64)
    np.cumsum(n1[:-1], out=start1[1:])
    start1 += C2

    # straddlers first within each (core, bin) group
    order = np.argsort(key * 2 + (1 - f_str.astype(np.int64)), kind="stable")
    key_s = key[order]
    group_start = np.searchsorted(key_s, key_s)     # first occurrence index
    rank = np.arange(key_s.shape[0]) - group_start
    j = rank // CHUNK
    bo = f_bin[order]
    col = np.where(j < n2[bo], start2[bo] + j, start1[bo] + (j - n2[bo]))
    lane = rank % CHUNK
    core_s = f_core[order]

    # per-entry precomputed device values (offsets use the entry's column)
    colmod_x = (col % AXSEG).astype(np.float64)
    colmod_y = (col % AYSEG).astype(np.float64)
    gx_e = (gx32[f_pid[order]]
            + (1.0 - 32.0 * f_xblk[order] + 36.0 * colmod_x).astype(f32)).astype(f32)
    gy_e = (gy32[f_pid[order]]
            + (1.0 - 4.0 * WY * f_yb[order] + 4.0 * WY * colmod_y).astype(f32)
            ).astype(f32)
    fw_e = (flux_v[f_pid[order]] / f32(64.0)).astype(np.float16)
    tv0_e = f_tv0[order].astype(np.float16)
    tv1_e = f_tv1[order].astype(np.float16)

    # per-column pad base values (benign: fw/tv pads are zero)
    colidx = np.arange(C)
    base_gx = (1.0 - 32.0 * chunk_xblk + 36.0 * (colidx % AXSEG)).astype(f32)
    base_gy = (1.0 - 4.0 * WY * chunk_yb + 4.0 * WY * (colidx % AYSEG)).astype(f32)

    per_core = []
    for k in range(N_CORES):
        m = core_s == k
        cols_k = col[m]
        lanes_k = lane[m]

        a_gx = np.empty((C, CHUNK), f32)
        a_gy = np.empty((C, CHUNK), f32)
        a_gx[:] = base_gx[:, None]
        a_gy[:] = base_gy[:, None]
        a_fw = np.zeros((C, CHUNK), np.float16)
        a_tv0 = np.zeros((C, CHUNK), np.float16)
        a_tv1 = np.zeros((C, CHUNK), np.float16)
        a_gx[cols_k, lanes_k] = gx_e[m]
        a_gy[cols_k, lanes_k] = gy_e[m]
        a_fw[cols_k, lanes_k] = fw_e[m]
        a_tv0[cols_k, lanes_k] = tv0_e[m]
        a_tv1[cols_k, lanes_k] = tv1_e[m]

        # pack per (lane, col): [gx f32 | gy f32 | fw | tv0 | tv1 | pad] u16x8
        pk = np.zeros((CHUNK, C, 8), np.uint16)
        pk[:, :, 0:2] = np.ascontiguousarray(a_gx.T).view(np.uint16).reshape(
            CHUNK, C, 2)
        pk[:, :, 2:4] = np.ascontiguousarray(a_gy.T).view(np.uint16).reshape(
            CHUNK, C, 2)
        pk[:, :, 4] = np.ascontiguousarray(a_fw.T).view(np.uint16)
        pk[:, :, 5] = np.ascontiguousarray(a_tv0.T).view(np.uint16)
        pk[:, :, 6] = np.ascontiguousarray(a_tv1.T).view(np.uint16)
        per_core.append({"pk": pk})

    return per_core, {"n_real_cols": C, "c2": C2}, chunk_tbl, C


# ---------------- device kernel ----------------
def build_kernel(C, chunk_tbl, num_devices=N_CORES, mm_bf16=True, n_real_cols=None,
                 c2=None):
    f = mybir.dt.float32
    h = mybir.dt.float16
    bf = mybir.dt.bfloat16
    if n_real_cols is None:
        n_real_cols = C
    if c2 is None:
        c2 = C
    nc = bacc.Bacc("TRN2", target_bir_lowering=False, debug=False,
                   enable_asserts=False, num_devices=num_devices)
    u16 = mybir.dt.uint16
    d_pk = nc.dram_tensor("pk", [CHUNK, C, 8], u16, kind="ExternalInput")
    d_out = [nc.dram_tensor(f"out{p}", [CHUNK, PLANES * N_PIX_LO], f,
                            kind="ExternalOutput") for p in range(2)]

    with tile.TileContext(nc) as tc, ExitStack() as ctx:
        pool = ctx.enter_context(tc.tile_pool(name="sbuf", bufs=1))
        aypool = ctx.enter_context(tc.tile_pool(name="ay", bufs=3))
        axpool = ctx.enter_context(tc.tile_pool(name="ax", bufs=3))
        ppool = ctx.enter_context(tc.tile_pool(name="psum", bufs=1, space="PSUM"))

        tpk = pool.tile([CHUNK, C, 8], u16, tag="tpk")

        zl = pool.tile([CHUNK, CHUNK], bf, tag="zl")
        zr = pool.tile([CHUNK, 512], bf, tag="zr")
        nc.vector.memset(zl[:], 0.0)
        nc.vector.memset(zr[:], 0.0)

        # two psum images (y-block parity); 8 plane strips + 1 dumpster each
        imgs = [ppool.tile([CHUNK, PLANES + 1, N_PIX_LO], f, tag=f"img{p}",
                           space="PSUM", name=f"img{p}") for p in range(2)]
        for img in imgs:
            nc.tensor.matmul(out=img[:, 0:4, :], lhsT=zl[:], rhs=zr[:],
                             start=True, stop=False)
            nc.tensor.matmul(out=img[:, 4:8, :], lhsT=zl[:], rhs=zr[:],
                             start=True, stop=False)
            nc.tensor.matmul(out=img[:, 8:9, :], lhsT=zl[:], rhs=zr[:, 0:128],
                             start=True, stop=False)

        def vw(nm, asl, an, w):
            off = {"gx": (0, 2, f), "gy": (2, 4, f), "fw": (4, 5, h),
                   "tv0": (5, 6, h), "tv1": (6, 7, h)}[nm]
            v = tpk[:, asl, off[0]:off[1]].bitcast(off[2])
            return v.to_broadcast([CHUNK, an, w])

        for g0 in range(0, C, GRP):
            gn = min(GRP, C - g0)
            sl = slice(g0, g0 + gn)
            nc.sync.dma_start(out=tpk[:, sl, :], in_=d_pk.ap()[:, sl, :])
            n2g = max(0, min(gn, c2 - g0))          # 2-slot cols in this group

            ay = aypool.tile([CHUNK, GRP, WY], bf, tag="ay")
            for a0 in range(0, gn, AYSEG):
                an = min(AYSEG, gn - a0)
                asl = slice(g0 + a0, g0 + a0 + an)
                nc.vector._custom_dve(
                    TRAP_OP, out=ay[:, a0:a0 + an, :],
                    in0=vw("fw", asl, an, WY),
                    in1=vw("gy", asl, an, WY),
                    s1=4.0)
            axp = axpool.tile([CHUNK, GRP, 2, WX], bf, tag="axp")
            for a0 in range(0, gn, AXSEG):
                an = min(AXSEG, gn - a0)
                asl = slice(g0 + a0, g0 + a0 + an)
                nc.vector._custom_dve(
                    TRAP_OP, out=axp[:, a0:a0 + an, 0, :],
                    in0=vw("tv0", asl, an, WX),
                    in1=vw("gx", asl, an, WX),
                    s1=4.0)
            for a0 in range(0, n2g, AXSEG):
                an = min(AXSEG, n2g - a0)
                asl = slice(g0 + a0, g0 + a0 + an)
                nc.vector._custom_dve(
                    TRAP_OP, out=axp[:, a0:a0 + an, 1, :],
                    in0=vw("tv1", asl, an, WX),
                    in1=vw("gx", asl, an, WX),
                    s1=4.0)

            for c in range(g0, min(g0 + gn, n_real_cols)):
                plane, yb, xblk = (int(chunk_tbl[c, 0]),
                                   int(chunk_tbl[c, 1]),
                                   int(chunk_tbl[c, 2]))
                par = yb & 1
                r = yb >> 1
                wx = min(WX, N_PIX_LO - xblk * 8)
                nsl = 2 if c < c2 else 1
                nc.tensor.matmul(
                    out=imgs[par][32 * r:32 * r + WY, plane:plane + nsl,
                                  xblk * 8:xblk * 8 + wx],
                    lhsT=ay[:, c - g0, :],
                    rhs=axp[:, c - g0, 0:nsl, 0:wx],
                    start=False, stop=False,
                    tile_position=(0, 32 * r))

        for img in imgs:
            nc.tensor.matmul(out=img[:, 0:4, :], lhsT=zl[:], rhs=zr[:],
                             start=False, stop=True)
            nc.tensor.matmul(out=img[:, 4:8, :], lhsT=zl[:], rhs=zr[:],
                             start=False, stop=True)
            nc.tensor.matmul(out=img[:, 8:9, :], lhsT=zl[:], rhs=zr[:, 0:128],
                             start=False, stop=True)

        for p in range(2):
            ot = pool.tile([CHUNK, PLANES * N_PIX_LO], f, tag=f"ot{p}")
            nc.scalar.copy(out=ot[:], in_=imgs[p][:, 0:PLANES, :])
            nc.sync.dma_start(out=d_out[p].ap(), in_=ot[:])

    nc.compile()
    return nc


def assemble(results):
    cube = np.empty((NV_LO, N_PIX_LO, N_PIX_LO), np.float32)
    for k in range(N_CORES):
        # img_par partition 32r+m (m<16) holds y cell 32r + 16*par + m
        for par in range(2):
            res = results[k][f"out{par}"].reshape(4, 32, PLANES, N_PIX_LO)
            # res[r, m] valid only for m < 16
            sub = res[:, 0:16]                      # [4, 16, PLANES, 128]
            for r in range(4):
                y0 = 32 * r + 16 * par
                cube[k * PLANES:(k + 1) * PLANES, y0:y0 + 16] = (
                    sub[r].transpose(1, 0, 2))
    return cube


# ---------------- entry point ----------------
def kernel(ra, dec, vel, flux):
    per_core, consts, chunk_tbl, C = route_points(ra, dec, vel, flux)
    if C == 0:  # no valid points at all
        return np.zeros((NV_LO, N_PIX_LO, N_PIX_LO), np.float32)
    _log(f"C={C} columns ({C * CHUNK} entry slots)")
    nc = build_kernel(C, chunk_tbl, n_real_cols=consts["n_real_cols"],
                      c2=consts["c2"])
    in_maps = [dict(per_core[k]) for k in range(N_CORES)]
    res = run_bass_kernel_spmd(nc, in_maps, core_ids=list(range(N_CORES)))
    return assemble(res.results)


# revision 25
# speedup vs baseline: 1.2493x; 1.0489x over previous
"""CloudRasterizerOversample Trainium2 kernel (v3).

Strategy
--------
Splat + 4x4x4 mean-pool is linear, so the pooled 64x128x128 cube is
built directly: the weight of a point to a lo-res cell along one axis
is a trapezoid t(u) = relu(min(u, 5-u, 1)) (u = g - 4c + 1) with
support on at most 2 consecutive cells.

Sharding: core k owns v-planes 8k..8k+7.  A point contributes to <=2
v-planes (p, p+1); one entry carries BOTH plane weights (tv0, tv1) and
the matmul writes both plane strips of PSUM in a single instruction
via a 2-block strided output access pattern (plane 7 spills into a
dumpster strip; the p+1 contribution is re-routed to the next core as
a duplicate entry).

Within a core, entries are binned by (plane, y-block of 16 cells,
x-block of 8).  Because a 16-wide weight tile must sit at a 32-aligned
PE column, PSUM holds TWO images (even/odd y-block parity); y-block
2r+par lands at partition base 32r of image par.  Entries that touch
only one v-plane (~75%) are packed into 1-slot columns whose matmul
streams 9 columns instead of 18 and whose slot-1 trapezoid is skipped;
straddling entries are sorted into a per-group prefix of 2-slot
columns.  Columns are ordered parity-major so image 0 closes, copies,
and DMAs out while image 1's matmuls still run.  Per 128-column group
the DVE builds, with one fused trapezoid instruction per segment,
    AY[128, 64, 16] = fw * t_y       (fw = flux/64)
    AX[128, 128, 2, 9]: slot s = tv_s * t_x
and per column one PE matmul accumulates
    img_par[32r:+16, (pl:pl+nsl)*128 + xblk*8 +: 9] += AY_c^T @ AX_c
All host-side address math (grid coords, per-segment scan offsets) is
precomputed into the single packed 16-byte-per-slot DMA tensor, so the
device only runs TRAP + MM.
"""

import os
import sys
import numpy as np
from contextlib import ExitStack

import concourse.bass as bass
import concourse.bacc as bacc
import concourse.mybir as mybir
import concourse.tile as tile
from concourse.bass_utils import run_bass_kernel_spmd

# ---------------- problem constants (hardcoded per spec) ----------------
N_PIX_LO = 128
OV_XY = 4
OV_V = 4
NV_LO = 64
PIX_LO = 0.1
VEL0_LO = -400.0
DV_LO = 12.5
N_PIX_HI = N_PIX_LO * OV_XY            # 512
PIX_HI = PIX_LO / OV_XY                # 0.025
FOV_HALF_HI = 0.5 * (N_PIX_HI - 1) * PIX_HI
DV_HI = DV_LO / OV_V                   # 3.125
VEL0_HI = VEL0_LO - 0.5 * (DV_LO - DV_HI)
NV_HI = NV_LO * OV_V                   # 256

N_CORES = 8
PLANES = NV_LO // N_CORES              # 8 base v-planes per core
NYB = 8                                # y blocks of 16 cells
NXB = 16                               # x blocks of 8 cells
WY = 16                                # y window width
WX = 9                                 # x window width
GRP = 128                              # columns per compute group
GRP0 = 32                              # size of the first (pipeline-fill) group


def _groups(C):
    out = []
    g0 = 0
    while g0 < C:
        gn = min(GRP0 if g0 == 0 else GRP, C - g0)
        out.append((g0, gn))
        g0 += gn
    return out
AYSEG = 128                            # columns per ay DVE call (2048 elems)
AXSEG = 128                            # columns per ax DVE call (1152 elems)
CHUNK = 128
NBINS = PLANES * NYB * NXB             # 1024 bins per core

# device scalars (f32)
INV_P = float(np.float32(1.0 / PIX_HI))
OFF_P = float(np.float32(FOV_HALF_HI / PIX_HI))
INV_DV = float(np.float32(1.0 / DV_HI))
VOFF = float(np.float32(-VEL0_HI / DV_HI))

_DBG = os.environ.get("KERNEL_DEBUG", "") != ""


def _log(*a):
    if _DBG:
        print("[kernel]", *a, file=sys.stderr, flush=True)


# ---------------- custom DVE op ----------------
from concourse.dve_spec import (
    Spec, Src0, Src1, C0, C1, Zero, One, AluOp, Bin, relu, minn, lower, scan,
)
from concourse.dve_ops import DveOp, OPS, CUSTOM_DVE_SPECS, _SUB_OPCODE_FOR_NAME
from concourse.dve_uop import DveOpSpec


def _trap_ref(in0, in1, c0, c1, c2):
    """out = in0 * relu(min(min(v, (1-v)+4), 1)), v = in1 - 4*Idx (global)."""
    in0 = np.asarray(in0, np.float32)
    in1 = np.asarray(in1, np.float32)
    n = int(np.prod(in0.shape[1:]))
    scan4 = (np.arange(n, dtype=np.float32) * np.float32(4.0)).reshape(in0.shape[1:])
    v = (in1 - scan4[None]).astype(np.float32)
    b = ((np.float32(1.0) - v) + np.float32(4.0)).astype(np.float32)
    m = np.minimum(np.minimum(v, b), np.float32(1.0))
    r = np.maximum(m, np.float32(0.0))
    return (in0 * r).astype(np.float32)


_scan4 = scan(AluOp.ADD, C1, init=Bin(AluOp.SUBTRACT, Zero, C1))
_v = Src1 - _scan4
TRAP_SPEC = Spec(body=Src0 * relu(minn(minn(_v, (One - _v) + C1), One)),
                 reference=_trap_ref)


def _mk_op(name, spec):
    if name in _SUB_OPCODE_FOR_NAME:
        for op in OPS:
            if op.name == name:
                return op
    shas = {}
    for ver in ("v3", "v4"):
        uops = lower(spec, ver=ver)
        row = max(_SUB_OPCODE_FOR_NAME.values()) + 1
        shas[ver] = DveOpSpec(name=name, opcode=row, uops=uops, rd1_en=True).sha(ver)
    op = DveOp(name, spec, subdim=False, uops_sha=shas)
    OPS.append(op)
    _SUB_OPCODE_FOR_NAME[name] = max(_SUB_OPCODE_FOR_NAME.values()) + 1
    CUSTOM_DVE_SPECS[name] = spec
    return op


TRAP_OP = _mk_op("RAST_TRAP_ANT", TRAP_SPEC)


# ---------------- host-side routing ----------------
def route_points(ra, dec, vel, flux):
    """Shard points by base v-plane across cores; bin by (plane, yblk, xblk).

    Returns (per_core [list of dict name->np array], consts dict (empty),
    chunk_tbl [C,3] int array of (plane, yblk, xblk), C).
    """
    f32 = np.float32
    ra = np.asarray(ra, f32)
    dec = np.asarray(dec, f32)
    vel = np.asarray(vel, f32)
    flux = np.asarray(flux, f32)

    # validity, exactly as the reference computes it (f32 add, f32 divide)
    def ref_idx(arr, off, scale):
        q = ((arr + f32(off)) / f32(scale)).astype(f32)
        return np.floor(q).astype(np.int64)

    ix0 = ref_idx(ra, FOV_HALF_HI, PIX_HI)
    iy0 = ref_idx(dec, FOV_HALF_HI, PIX_HI)
    iv0 = ref_idx(vel, -VEL0_HI, DV_HI)
    valid = ((ix0 >= 0) & (ix0 < N_PIX_HI - 1) &
             (iy0 >= 0) & (iy0 < N_PIX_HI - 1) &
             (iv0 >= 0) & (iv0 < NV_HI - 1))

    ra_v = ra[valid]
    dec_v = dec[valid]
    vel_v = vel[valid]
    flux_v = flux[valid]

    # device-order grid coords (f32 mult + add); f64 copies for exact floors
    gx32 = (ra_v * f32(INV_P) + f32(OFF_P)).astype(f32)
    gy32 = (dec_v * f32(INV_P) + f32(OFF_P)).astype(f32)
    gv32 = (vel_v * f32(INV_DV) + f32(VOFF)).astype(f32)
    gxd = gx32.astype(np.float64)
    gyd = gy32.astype(np.float64)
    gvd = gv32.astype(np.float64)

    cx = (np.floor((gxd - 4.0) / 4.0) + 1).astype(np.int64)
    cy = (np.floor((gyd - 4.0) / 4.0) + 1).astype(np.int64)
    cv = (np.floor((gvd - 4.0) / 4.0) + 1).astype(np.int64)
    np.clip(cx, 0, N_PIX_LO - 1, out=cx)
    np.clip(cy, 0, N_PIX_LO - 1, out=cy)
    np.clip(cv, 0, NV_LO - 1, out=cv)

    def trapv(u):
        m = np.minimum(np.minimum(u + f32(1.0), f32(4.0) - u), f32(1.0))
        return np.maximum(m, f32(0.0)).astype(f32)

    tv0 = trapv(gv32 - (4.0 * cv).astype(f32))
    tv1 = trapv(gv32 - (4.0 * (cv + 1)).astype(f32))

    n = ra_v.shape[0]
    pid = np.arange(n)

    # v core-boundary duplication: base plane local 7 with spill into the
    # next core's plane 0
    bdup = ((cv & 7) == 7) & (tv1 > 0)
    e_pid = np.concatenate([pid, pid[bdup]])
    e_pl = np.concatenate([cv, cv[bdup] + 1])       # global base plane
    e_tv0 = np.concatenate([tv0, tv1[bdup]])
    e_tv1 = np.concatenate([tv1, np.zeros(int(bdup.sum()), f32)])

    # y duplication at y-block boundary
    e_cy = cy[e_pid]
    sy = gyd[e_pid] > 4.0 * e_cy + 3.0              # t(cy+1) > 0
    ydup = sy & ((e_cy & (WY - 1)) == WY - 1) & (e_cy < N_PIX_LO - 1)
    f_pid = np.concatenate([e_pid, e_pid[ydup]])
    f_pl = np.concatenate([e_pl, e_pl[ydup]])
    f_tv0 = np.concatenate([e_tv0, e_tv0[ydup]])
    f_tv1 = np.concatenate([e_tv1, e_tv1[ydup]])
    f_yb = np.concatenate([e_cy >> 4, (e_cy[ydup] >> 4) + 1])

    f_xblk = cx[f_pid] >> 3
    f_core = f_pl >> 3
    f_plane = f_pl & 7
    f_bin = (f_plane * NYB + f_yb) * NXB + f_xblk
    f_str = f_tv1 > 0                               # needs the 2nd v-plane slot

    key = f_core * NBINS + f_bin
    counts = np.bincount(key, minlength=N_CORES * NBINS).reshape(N_CORES, NBINS)
    scount = np.bincount(key[f_str], minlength=N_CORES * NBINS).reshape(
        N_CORES, NBINS)
    maxc = counts.max(axis=0)
    nchunks = (maxc + CHUNK - 1) // CHUNK           # 0 for empty bins
    n2 = (scount.max(axis=0) + CHUNK - 1) // CHUNK  # 2-slot chunks per bin
    n1 = nchunks - n2

    # chunk table (shared across cores); 2-slot columns first globally
    plane_b, rem = np.divmod(np.arange(NBINS), NYB * NXB)
    yb_b, xblk_b = np.divmod(rem, NXB)
    chunk_plane = np.concatenate([np.repeat(plane_b, n2), np.repeat(plane_b, n1)])
    chunk_yb = np.concatenate([np.repeat(yb_b, n2), np.repeat(yb_b, n1)])
    chunk_xblk = np.concatenate([np.repeat(xblk_b, n2), np.repeat(xblk_b, n1)])
    C2 = int(n2.sum())
    C = chunk_plane.shape[0]
    chunk_tbl = np.stack([chunk_plane, chunk_yb, chunk_xblk], axis=1)

    start2 = np.zeros(NBINS, np.int64)
    np.cumsum(n2[:-1], out=start2[1:])
    start1 = np.zeros(NBINS, np.int64)
    np.cumsum(n1[:-1], out=start1[1:])
    start1 += C2

    # straddlers first within each (core, bin) group
    order = np.argsort(key * 2 + (1 - f_str.astype(np.int64)), kind="stable")
    key_s = key[order]
    group_start = np.searchsorted(key_s, key_s)     # first occurrence index
    rank = np.arange(key_s.shape[0]) - group_start
    j = rank // CHUNK
    bo = f_bin[order]
    col = np.where(j < n2[bo], start2[bo] + j, start1[bo] + (j - n2[bo]))
    lane = rank % CHUNK
    core_s = f_core[order]

    # Reorder columns: parity-major (all even y-block columns first, so the
    # even psum image can close/copy/DMA while odd-parity matmuls still run),
    # and within each parity spread the 2-slot columns evenly across groups
    # (each group = [2-slot share | 1-slot share]).
    par_of = (chunk_tbl[:, 1] & 1).astype(np.int64)
    is2 = np.arange(C) < C2
    groups = _groups(C)
    seq = []                                        # parity-major merged order
    for p in (0, 1):
        l2 = list(np.nonzero((par_of == p) & is2)[0])
        l1 = list(np.nonzero((par_of == p) & ~is2)[0])
        na, nb = len(l2), len(l1)
        i2 = i1 = 0
        for t in range(na + nb):                    # even interleave of l2
            if i2 < na and (i1 >= nb or i2 * (na + nb) <= t * na):
                seq.append(l2[i2])
                i2 += 1
            else:
                seq.append(l1[i1])
                i1 += 1
    new_order = []
    n2g = []
    for g0, gn in groups:
        take = seq[g0:g0 + gn]
        take2 = [c for c in take if c < C2]         # 2-slot prefix per group
        take1 = [c for c in take if c >= C2]
        new_order += take2 + take1
        n2g.append(len(take2))
    new_order = np.asarray(new_order, np.int64)
    old2new = np.empty(C, np.int64)
    old2new[new_order] = np.arange(C)
    col = old2new[col]
    chunk_tbl = chunk_tbl[new_order]
    n2g = np.asarray(n2g, np.int64)

    # per-entry precomputed device values: scan offsets are relative to the
    # DVE segment start of the entry's column (segments follow _groups)
    segx = np.empty(C, np.int64)
    segy = np.empty(C, np.int64)
    for g0, gn in groups:
        for a0 in range(0, gn, AXSEG):
            segx[g0 + a0:g0 + a0 + min(AXSEG, gn - a0)] = g0 + a0
        for a0 in range(0, gn, AYSEG):
            segy[g0 + a0:g0 + a0 + min(AYSEG, gn - a0)] = g0 + a0
    colmod_x = (col - segx[col]).astype(np.float64)
    colmod_y = (col - segy[col]).astype(np.float64)
    gx_e = (gx32[f_pid[order]]
            + (1.0 - 32.0 * f_xblk[order] + 36.0 * colmod_x).astype(f32)).astype(f32)
    gy_e = (gy32[f_pid[order]]
            + (1.0 - 4.0 * WY * f_yb[order] + 4.0 * WY * colmod_y).astype(f32)
            ).astype(f32)
    fw_e = (flux_v[f_pid[order]] / f32(64.0)).astype(np.float16)
    tv0_e = f_tv0[order].astype(np.float16)
    tv1_e = f_tv1[order].astype(np.float16)

    # per-column pad base values (benign: fw/tv pads are zero); note
    # chunk_plane/chunk_yb/chunk_xblk are in OLD column order while the pad
    # base must follow the NEW order -> use the permuted chunk_tbl
    colidx = np.arange(C)
    base_gx = (1.0 - 32.0 * chunk_tbl[:, 2] + 36.0 * (colidx - segx)).astype(f32)
    base_gy = (1.0 - 4.0 * WY * chunk_tbl[:, 1]
               + 4.0 * WY * (colidx - segy)).astype(f32)

    per_core = []
    for k in range(N_CORES):
        m = core_s == k
        cols_k = col[m]
        lanes_k = lane[m]

        a_gx = np.empty((C, CHUNK), f32)
        a_gy = np.empty((C, CHUNK), f32)
        a_gx[:] = base_gx[:, None]
        a_gy[:] = base_gy[:, None]
        a_fw = np.zeros((C, CHUNK), np.float16)
        a_tv0 = np.zeros((C, CHUNK), np.float16)
        a_tv1 = np.zeros((C, CHUNK), np.float16)
        a_gx[cols_k, lanes_k] = gx_e[m]
        a_gy[cols_k, lanes_k] = gy_e[m]
        a_fw[cols_k, lanes_k] = fw_e[m]
        a_tv0[cols_k, lanes_k] = tv0_e[m]
        a_tv1[cols_k, lanes_k] = tv1_e[m]

        # pack per (lane, col): [gx f32 | gy f32 | fw | tv0 | tv1 | pad] u16x8
        pk = np.zeros((CHUNK, C, 8), np.uint16)
        pk[:, :, 0:2] = np.ascontiguousarray(a_gx.T).view(np.uint16).reshape(
            CHUNK, C, 2)
        pk[:, :, 2:4] = np.ascontiguousarray(a_gy.T).view(np.uint16).reshape(
            CHUNK, C, 2)
        pk[:, :, 4] = np.ascontiguousarray(a_fw.T).view(np.uint16)
        pk[:, :, 5] = np.ascontiguousarray(a_tv0.T).view(np.uint16)
        pk[:, :, 6] = np.ascontiguousarray(a_tv1.T).view(np.uint16)
        per_core.append({"pk": pk})

    return per_core, {"n_real_cols": C, "c2": C2, "n2g": n2g}, chunk_tbl, C


# ---------------- device kernel ----------------
def build_kernel(C, chunk_tbl, num_devices=N_CORES, mm_bf16=True, n_real_cols=None,
                 c2=None, n2g=None):
    f = mybir.dt.float32
    h = mybir.dt.float16
    bf = mybir.dt.bfloat16
    if n_real_cols is None:
        n_real_cols = C
    if n2g is None:
        n2g = [GRP] * ((C + GRP - 1) // GRP)
    nc = bacc.Bacc("TRN2", target_bir_lowering=False, debug=False,
                   enable_asserts=False, num_devices=num_devices)
    u16 = mybir.dt.uint16
    d_pk = nc.dram_tensor("pk", [CHUNK, C, 8], u16, kind="ExternalInput")
    d_out = [nc.dram_tensor(f"out{p}", [CHUNK, PLANES * N_PIX_LO], f,
                            kind="ExternalOutput") for p in range(2)]

    with tile.TileContext(nc) as tc, ExitStack() as ctx:
        pool = ctx.enter_context(tc.tile_pool(name="sbuf", bufs=1))
        aypool = ctx.enter_context(tc.tile_pool(name="ay", bufs=6))
        axpool = ctx.enter_context(tc.tile_pool(name="ax", bufs=6))
        ppool = ctx.enter_context(tc.tile_pool(name="psum", bufs=1, space="PSUM"))

        tpk = pool.tile([CHUNK, C, 8], u16, tag="tpk")

        zl = pool.tile([CHUNK, CHUNK], bf, tag="zl")
        zr = pool.tile([CHUNK, 512], bf, tag="zr")
        nc.gpsimd.memset(zl[:], 0.0)
        nc.gpsimd.memset(zr[:], 0.0)

        # two psum images (y-block parity); 8 plane strips + 1 dumpster each
        imgs = [ppool.tile([CHUNK, PLANES + 1, N_PIX_LO], f, tag=f"img{p}",
                           space="PSUM", name=f"img{p}") for p in range(2)]
        for img in imgs:
            nc.tensor.matmul(out=img[:, 0:4, :], lhsT=zl[:], rhs=zr[:],
                             start=True, stop=False)
            nc.tensor.matmul(out=img[:, 4:8, :], lhsT=zl[:], rhs=zr[:],
                             start=True, stop=False)
            nc.tensor.matmul(out=img[:, 8:9, :], lhsT=zl[:], rhs=zr[:, 0:128],
                             start=True, stop=False)

        def vw(nm, asl, an, w):
            off = {"gx": (0, 2, f), "gy": (2, 4, f), "fw": (4, 5, h),
                   "tv0": (5, 6, h), "tv1": (6, 7, h)}[nm]
            v = tpk[:, asl, off[0]:off[1]].bitcast(off[2])
            return v.to_broadcast([CHUNK, an, w])

        glist = _groups(C)
        last_par_group = {0: -1, 1: -1}
        for g, (g0, gn) in enumerate(glist):
            pars = set(int(chunk_tbl[c, 1]) & 1
                       for c in range(g0, min(g0 + gn, n_real_cols)))
            for p in pars:
                last_par_group[p] = g

        def close_img(p):
            img = imgs[p]
            nc.tensor.matmul(out=img[:, 0:4, :], lhsT=zl[:], rhs=zr[:],
                             start=False, stop=True)
            nc.tensor.matmul(out=img[:, 4:8, :], lhsT=zl[:], rhs=zr[:],
                             start=False, stop=True)
            nc.tensor.matmul(out=img[:, 8:9, :], lhsT=zl[:], rhs=zr[:, 0:128],
                             start=False, stop=True)
            ot = pool.tile([CHUNK, PLANES * N_PIX_LO], f, tag=f"ot{p}",
                           name=f"ot{p}")
            nc.scalar.copy(out=ot[:], in_=img[:, 0:PLANES, :])
            nc.sync.dma_start(out=d_out[p].ap(), in_=ot[:])

        for g, (g0, gn) in enumerate(glist):
            sl = slice(g0, g0 + gn)
            nc.sync.dma_start(out=tpk[:, sl, :], in_=d_pk.ap()[:, sl, :])
            k2 = int(n2g[g])                        # 2-slot cols in this group

            # x slots first, then per ay segment: trapezoid + its matmuls
            axp = axpool.tile([CHUNK, GRP, 2, WX], bf, tag="axp")
            for a0 in range(0, gn, AXSEG):
                an = min(AXSEG, gn - a0)
                asl = slice(g0 + a0, g0 + a0 + an)
                nc.vector._custom_dve(
                    TRAP_OP, out=axp[:, a0:a0 + an, 0, :],
                    in0=vw("tv0", asl, an, WX),
                    in1=vw("gx", asl, an, WX),
                    s1=4.0)
            for a0 in range(0, k2, AXSEG):
                an = min(AXSEG, k2 - a0)
                asl = slice(g0 + a0, g0 + a0 + an)
                nc.vector._custom_dve(
                    TRAP_OP, out=axp[:, a0:a0 + an, 1, :],
                    in0=vw("tv1", asl, an, WX),
                    in1=vw("gx", asl, an, WX),
                    s1=4.0)

            ay = aypool.tile([CHUNK, GRP, WY], bf, tag="ay")
            for a0 in range(0, gn, AYSEG):
                an = min(AYSEG, gn - a0)
                asl = slice(g0 + a0, g0 + a0 + an)
                nc.vector._custom_dve(
                    TRAP_OP, out=ay[:, a0:a0 + an, :],
                    in0=vw("fw", asl, an, WY),
                    in1=vw("gy", asl, an, WY),
                    s1=4.0)
                for c in range(g0 + a0, min(g0 + a0 + an, n_real_cols)):
                    plane, yb, xblk = (int(chunk_tbl[c, 0]),
                                       int(chunk_tbl[c, 1]),
                                       int(chunk_tbl[c, 2]))
                    par = yb & 1
                    r = yb >> 1
                    wx = min(WX, N_PIX_LO - xblk * 8)
                    nsl = 2 if (c - g0) < k2 else 1
                    nc.tensor.matmul(
                        out=imgs[par][32 * r:32 * r + WY, plane:plane + nsl,
                                      xblk * 8:xblk * 8 + wx],
                        lhsT=ay[:, c - g0, :],
                        rhs=axp[:, c - g0, 0:nsl, 0:wx],
                        start=False, stop=False,
                        tile_position=(0, 32 * r))
            for p in (0, 1):
                if last_par_group[p] == g:
                    close_img(p)

        for p in (0, 1):
            if last_par_group[p] == -1:             # parity never touched
                close_img(p)

    nc.compile()
    return nc


def assemble(results):
    cube = np.empty((NV_LO, N_PIX_LO, N_PIX_LO), np.float32)
    for k in range(N_CORES):
        # img_par partition 32r+m (m<16) holds y cell 32r + 16*par + m
        for par in range(2):
            res = results[k][f"out{par}"].reshape(4, 32, PLANES, N_PIX_LO)
            # res[r, m] valid only for m < 16
            sub = res[:, 0:16]                      # [4, 16, PLANES, 128]
            for r in range(4):
                y0 = 32 * r + 16 * par
                cube[k * PLANES:(k + 1) * PLANES, y0:y0 + 16] = (
                    sub[r].transpose(1, 0, 2))
    return cube


# ---------------- entry point ----------------
def kernel(ra, dec, vel, flux):
    per_core, consts, chunk_tbl, C = route_points(ra, dec, vel, flux)
    if C == 0:  # no valid points at all
        return np.zeros((NV_LO, N_PIX_LO, N_PIX_LO), np.float32)
    _log(f"C={C} columns ({C * CHUNK} entry slots)")
    nc = build_kernel(C, chunk_tbl, n_real_cols=consts["n_real_cols"],
                      c2=consts["c2"], n2g=consts["n2g"])
    in_maps = [dict(per_core[k]) for k in range(N_CORES)]
    res = run_bass_kernel_spmd(nc, in_maps, core_ids=list(range(N_CORES)))
    return assemble(res.results)


# revision 28
# speedup vs baseline: 1.2549x; 1.0045x over previous
"""CloudRasterizerOversample Trainium2 kernel (v3).

Strategy
--------
Splat + 4x4x4 mean-pool is linear, so the pooled 64x128x128 cube is
built directly: the weight of a point to a lo-res cell along one axis
is a trapezoid t(u) = relu(min(u, 5-u, 1)) (u = g - 4c + 1) with
support on at most 2 consecutive cells.

Sharding: core k owns v-planes 8k..8k+7.  A point contributes to <=2
v-planes (p, p+1); one entry carries BOTH plane weights (tv0, tv1) and
the matmul writes both plane strips of PSUM in a single instruction
via a 2-block strided output access pattern (plane 7 spills into a
dumpster strip; the p+1 contribution is re-routed to the next core as
a duplicate entry).

Within a core, entries are binned by (plane, y-block of 16 cells,
x-block of 8).  Because a 16-wide weight tile must sit at a 32-aligned
PE column, PSUM holds TWO images (even/odd y-block parity); y-block
2r+par lands at partition base 32r of image par.  Entries that touch
only one v-plane (~75%) are packed into 1-slot columns whose matmul
streams 9 columns instead of 18 and whose slot-1 trapezoid is skipped;
straddling entries are sorted into a per-group prefix of 2-slot
columns.  Columns are ordered parity-major so image 0 closes, copies,
and DMAs out while image 1's matmuls still run.  Per 128-column group
the DVE builds, with one fused trapezoid instruction per segment,
    AY[128, 64, 16] = fw * t_y       (fw = flux/64)
    AX[128, 128, 2, 9]: slot s = tv_s * t_x
and per column one PE matmul accumulates
    img_par[32r:+16, (pl:pl+nsl)*128 + xblk*8 +: 9] += AY_c^T @ AX_c
All host-side address math (grid coords, per-segment scan offsets) is
precomputed into the single packed 16-byte-per-slot DMA tensor, so the
device only runs TRAP + MM.
"""

import os
import sys
import numpy as np
from contextlib import ExitStack

import concourse.bass as bass
import concourse.bacc as bacc
import concourse.mybir as mybir
import concourse.tile as tile
from concourse.bass_utils import run_bass_kernel_spmd

# ---------------- problem constants (hardcoded per spec) ----------------
N_PIX_LO = 128
OV_XY = 4
OV_V = 4
NV_LO = 64
PIX_LO = 0.1
VEL0_LO = -400.0
DV_LO = 12.5
N_PIX_HI = N_PIX_LO * OV_XY            # 512
PIX_HI = PIX_LO / OV_XY                # 0.025
FOV_HALF_HI = 0.5 * (N_PIX_HI - 1) * PIX_HI
DV_HI = DV_LO / OV_V                   # 3.125
VEL0_HI = VEL0_LO - 0.5 * (DV_LO - DV_HI)
NV_HI = NV_LO * OV_V                   # 256

N_CORES = 8
PLANES = NV_LO // N_CORES              # 8 base v-planes per core
NYB = 8                                # y blocks of 16 cells
NXB = 16                               # x blocks of 8 cells
WY = 16                                # y window width
WX = 9                                 # x window width
GRP = 128                              # columns per compute group
GRP0 = 32                              # size of the first (pipeline-fill) group


def _groups(C):
    out = []
    g0 = 0
    while g0 < C:
        gn = min(GRP0 if g0 == 0 else GRP, C - g0)
        out.append((g0, gn))
        g0 += gn
    return out
AYSEG = 128                            # columns per ay DVE call (2048 elems)
AXSEG = 128                            # columns per ax DVE call (1152 elems)
CHUNK = 128
NBINS = PLANES * NYB * NXB             # 1024 bins per core

# device scalars (f32)
INV_P = float(np.float32(1.0 / PIX_HI))
OFF_P = float(np.float32(FOV_HALF_HI / PIX_HI))
INV_DV = float(np.float32(1.0 / DV_HI))
VOFF = float(np.float32(-VEL0_HI / DV_HI))

_DBG = os.environ.get("KERNEL_DEBUG", "") != ""


def _log(*a):
    if _DBG:
        print("[kernel]", *a, file=sys.stderr, flush=True)


# ---------------- custom DVE op ----------------
from concourse.dve_spec import (
    Spec, Src0, Src1, C0, C1, Zero, One, AluOp, Bin, relu, minn, lower, scan,
)
from concourse.dve_ops import DveOp, OPS, CUSTOM_DVE_SPECS, _SUB_OPCODE_FOR_NAME
from concourse.dve_uop import DveOpSpec


def _trap_ref(in0, in1, c0, c1, c2):
    """out = in0 * relu(min(min(v, (1-v)+4), 1)), v = in1 - 4*Idx (global)."""
    in0 = np.asarray(in0, np.float32)
    in1 = np.asarray(in1, np.float32)
    n = int(np.prod(in0.shape[1:]))
    scan4 = (np.arange(n, dtype=np.float32) * np.float32(4.0)).reshape(in0.shape[1:])
    v = (in1 - scan4[None]).astype(np.float32)
    b = ((np.float32(1.0) - v) + np.float32(4.0)).astype(np.float32)
    m = np.minimum(np.minimum(v, b), np.float32(1.0))
    r = np.maximum(m, np.float32(0.0))
    return (in0 * r).astype(np.float32)


_scan4 = scan(AluOp.ADD, C1, init=Bin(AluOp.SUBTRACT, Zero, C1))
_v = Src1 - _scan4
TRAP_SPEC = Spec(body=Src0 * relu(minn(minn(_v, (One - _v) + C1), One)),
                 reference=_trap_ref)


def _mk_op(name, spec):
    if name in _SUB_OPCODE_FOR_NAME:
        for op in OPS:
            if op.name == name:
                return op
    shas = {}
    for ver in ("v3", "v4"):
        uops = lower(spec, ver=ver)
        row = max(_SUB_OPCODE_FOR_NAME.values()) + 1
        shas[ver] = DveOpSpec(name=name, opcode=row, uops=uops, rd1_en=True).sha(ver)
    op = DveOp(name, spec, subdim=False, uops_sha=shas)
    OPS.append(op)
    _SUB_OPCODE_FOR_NAME[name] = max(_SUB_OPCODE_FOR_NAME.values()) + 1
    CUSTOM_DVE_SPECS[name] = spec
    return op


TRAP_OP = _mk_op("RAST_TRAP_ANT", TRAP_SPEC)


# ---------------- host-side routing ----------------
def route_points(ra, dec, vel, flux):
    """Shard points by base v-plane across cores; bin by (plane, yblk, xblk).

    Returns (per_core [list of dict name->np array], consts dict (empty),
    chunk_tbl [C,3] int array of (plane, yblk, xblk), C).
    """
    f32 = np.float32
    ra = np.asarray(ra, f32)
    dec = np.asarray(dec, f32)
    vel = np.asarray(vel, f32)
    flux = np.asarray(flux, f32)

    # validity, exactly as the reference computes it (f32 add, f32 divide)
    def ref_idx(arr, off, scale):
        q = ((arr + f32(off)) / f32(scale)).astype(f32)
        return np.floor(q).astype(np.int64)

    ix0 = ref_idx(ra, FOV_HALF_HI, PIX_HI)
    iy0 = ref_idx(dec, FOV_HALF_HI, PIX_HI)
    iv0 = ref_idx(vel, -VEL0_HI, DV_HI)
    valid = ((ix0 >= 0) & (ix0 < N_PIX_HI - 1) &
             (iy0 >= 0) & (iy0 < N_PIX_HI - 1) &
             (iv0 >= 0) & (iv0 < NV_HI - 1))

    ra_v = ra[valid]
    dec_v = dec[valid]
    vel_v = vel[valid]
    flux_v = flux[valid]

    # device-order grid coords (f32 mult + add); f64 copies for exact floors
    gx32 = (ra_v * f32(INV_P) + f32(OFF_P)).astype(f32)
    gy32 = (dec_v * f32(INV_P) + f32(OFF_P)).astype(f32)
    gv32 = (vel_v * f32(INV_DV) + f32(VOFF)).astype(f32)
    gxd = gx32.astype(np.float64)
    gyd = gy32.astype(np.float64)
    gvd = gv32.astype(np.float64)

    cx = (np.floor((gxd - 4.0) / 4.0) + 1).astype(np.int64)
    cy = (np.floor((gyd - 4.0) / 4.0) + 1).astype(np.int64)
    cv = (np.floor((gvd - 4.0) / 4.0) + 1).astype(np.int64)
    np.clip(cx, 0, N_PIX_LO - 1, out=cx)
    np.clip(cy, 0, N_PIX_LO - 1, out=cy)
    np.clip(cv, 0, NV_LO - 1, out=cv)

    def trapv(u):
        m = np.minimum(np.minimum(u + f32(1.0), f32(4.0) - u), f32(1.0))
        return np.maximum(m, f32(0.0)).astype(f32)

    tv0 = trapv(gv32 - (4.0 * cv).astype(f32))
    tv1 = trapv(gv32 - (4.0 * (cv + 1)).astype(f32))

    n = ra_v.shape[0]
    pid = np.arange(n)

    # v core-boundary duplication: base plane local 7 with spill into the
    # next core's plane 0
    bdup = ((cv & 7) == 7) & (tv1 > 0)
    e_pid = np.concatenate([pid, pid[bdup]])
    e_pl = np.concatenate([cv, cv[bdup] + 1])       # global base plane
    e_tv0 = np.concatenate([tv0, tv1[bdup]])
    e_tv1 = np.concatenate([tv1, np.zeros(int(bdup.sum()), f32)])

    # y duplication at y-block boundary
    e_cy = cy[e_pid]
    sy = gyd[e_pid] > 4.0 * e_cy + 3.0              # t(cy+1) > 0
    ydup = sy & ((e_cy & (WY - 1)) == WY - 1) & (e_cy < N_PIX_LO - 1)
    f_pid = np.concatenate([e_pid, e_pid[ydup]])
    f_pl = np.concatenate([e_pl, e_pl[ydup]])
    f_tv0 = np.concatenate([e_tv0, e_tv0[ydup]])
    f_tv1 = np.concatenate([e_tv1, e_tv1[ydup]])
    f_yb = np.concatenate([e_cy >> 4, (e_cy[ydup] >> 4) + 1])

    f_xblk = cx[f_pid] >> 3
    f_core = f_pl >> 3
    f_plane = f_pl & 7
    f_bin = (f_plane * NYB + f_yb) * NXB + f_xblk
    f_str = f_tv1 > 0                               # needs the 2nd v-plane slot

    key = f_core * NBINS + f_bin
    counts = np.bincount(key, minlength=N_CORES * NBINS).reshape(N_CORES, NBINS)
    scount = np.bincount(key[f_str], minlength=N_CORES * NBINS).reshape(
        N_CORES, NBINS)
    maxc = counts.max(axis=0)
    nchunks = (maxc + CHUNK - 1) // CHUNK           # 0 for empty bins
    n2 = (scount.max(axis=0) + CHUNK - 1) // CHUNK  # 2-slot chunks per bin
    n1 = nchunks - n2

    # chunk table (shared across cores); 2-slot columns first globally
    plane_b, rem = np.divmod(np.arange(NBINS), NYB * NXB)
    yb_b, xblk_b = np.divmod(rem, NXB)
    chunk_plane = np.concatenate([np.repeat(plane_b, n2), np.repeat(plane_b, n1)])
    chunk_yb = np.concatenate([np.repeat(yb_b, n2), np.repeat(yb_b, n1)])
    chunk_xblk = np.concatenate([np.repeat(xblk_b, n2), np.repeat(xblk_b, n1)])
    C2 = int(n2.sum())
    C = chunk_plane.shape[0]
    chunk_tbl = np.stack([chunk_plane, chunk_yb, chunk_xblk], axis=1)

    start2 = np.zeros(NBINS, np.int64)
    np.cumsum(n2[:-1], out=start2[1:])
    start1 = np.zeros(NBINS, np.int64)
    np.cumsum(n1[:-1], out=start1[1:])
    start1 += C2

    # straddlers first within each (core, bin) group
    order = np.argsort(key * 2 + (1 - f_str.astype(np.int64)), kind="stable")
    key_s = key[order]
    group_start = np.searchsorted(key_s, key_s)     # first occurrence index
    rank = np.arange(key_s.shape[0]) - group_start
    j = rank // CHUNK
    bo = f_bin[order]
    col = np.where(j < n2[bo], start2[bo] + j, start1[bo] + (j - n2[bo]))
    lane = rank % CHUNK
    core_s = f_core[order]

    # Reorder columns: parity-major (all even y-block columns first, so the
    # even psum image can close/copy/DMA while odd-parity matmuls still run),
    # and within each parity spread the 2-slot columns evenly across groups
    # (each group = [2-slot share | 1-slot share]).
    par_of = (chunk_tbl[:, 1] & 1).astype(np.int64)
    is2 = np.arange(C) < C2
    groups = _groups(C)
    seq = []                                        # parity-major merged order
    for p in (0, 1):
        l2 = list(np.nonzero((par_of == p) & is2)[0])
        l1 = list(np.nonzero((par_of == p) & ~is2)[0])
        na, nb = len(l2), len(l1)
        i2 = i1 = 0
        for t in range(na + nb):                    # even interleave of l2
            if i2 < na and (i1 >= nb or i2 * (na + nb) <= t * na):
                seq.append(l2[i2])
                i2 += 1
            else:
                seq.append(l1[i1])
                i1 += 1
    new_order = []
    n2g = []
    for g0, gn in groups:
        take = seq[g0:g0 + gn]
        take2 = [c for c in take if c < C2]         # 2-slot prefix per group
        take1 = [c for c in take if c >= C2]
        new_order += take2 + take1
        n2g.append(len(take2))
    new_order = np.asarray(new_order, np.int64)
    old2new = np.empty(C, np.int64)
    old2new[new_order] = np.arange(C)
    col = old2new[col]
    chunk_tbl = chunk_tbl[new_order]
    n2g = np.asarray(n2g, np.int64)

    # per-entry precomputed device values: scan offsets are relative to the
    # DVE segment start of the entry's column (segments follow _groups)
    segx = np.empty(C, np.int64)
    segy = np.empty(C, np.int64)
    for g0, gn in groups:
        for a0 in range(0, gn, AXSEG):
            segx[g0 + a0:g0 + a0 + min(AXSEG, gn - a0)] = g0 + a0
        for a0 in range(0, gn, AYSEG):
            segy[g0 + a0:g0 + a0 + min(AYSEG, gn - a0)] = g0 + a0
    colmod_x = (col - segx[col]).astype(np.float64)
    colmod_y = (col - segy[col]).astype(np.float64)
    gx_e = (gx32[f_pid[order]]
            + (1.0 - 32.0 * f_xblk[order] + 36.0 * colmod_x).astype(f32)).astype(f32)
    gy_e = (gy32[f_pid[order]]
            + (1.0 - 4.0 * WY * f_yb[order] + 4.0 * WY * colmod_y).astype(f32)
            ).astype(f32)
    fw_e = (flux_v[f_pid[order]] / f32(64.0)).astype(np.float16)
    tv0_e = f_tv0[order].astype(np.float16)
    tv1_e = f_tv1[order].astype(np.float16)

    # per-column pad base values (benign: fw/tv pads are zero); note
    # chunk_plane/chunk_yb/chunk_xblk are in OLD column order while the pad
    # base must follow the NEW order -> use the permuted chunk_tbl
    colidx = np.arange(C)
    base_gx = (1.0 - 32.0 * chunk_tbl[:, 2] + 36.0 * (colidx - segx)).astype(f32)
    base_gy = (1.0 - 4.0 * WY * chunk_tbl[:, 1]
               + 4.0 * WY * (colidx - segy)).astype(f32)

    per_core = []
    for k in range(N_CORES):
        m = core_s == k
        cols_k = col[m]
        lanes_k = lane[m]

        a_gx = np.empty((C, CHUNK), f32)
        a_gy = np.empty((C, CHUNK), f32)
        a_gx[:] = base_gx[:, None]
        a_gy[:] = base_gy[:, None]
        a_fw = np.zeros((C, CHUNK), np.float16)
        a_tv0 = np.zeros((C, CHUNK), np.float16)
        a_tv1 = np.zeros((C, CHUNK), np.float16)
        a_gx[cols_k, lanes_k] = gx_e[m]
        a_gy[cols_k, lanes_k] = gy_e[m]
        a_fw[cols_k, lanes_k] = fw_e[m]
        a_tv0[cols_k, lanes_k] = tv0_e[m]
        a_tv1[cols_k, lanes_k] = tv1_e[m]

        # pack per (lane, col): [gx f32 | gy f32 | fw | tv0 | tv1 | pad] u16x8
        pk = np.zeros((CHUNK, C, 8), np.uint16)
        pk[:, :, 0:2] = np.ascontiguousarray(a_gx.T).view(np.uint16).reshape(
            CHUNK, C, 2)
        pk[:, :, 2:4] = np.ascontiguousarray(a_gy.T).view(np.uint16).reshape(
            CHUNK, C, 2)
        pk[:, :, 4] = np.ascontiguousarray(a_fw.T).view(np.uint16)
        pk[:, :, 5] = np.ascontiguousarray(a_tv0.T).view(np.uint16)
        pk[:, :, 6] = np.ascontiguousarray(a_tv1.T).view(np.uint16)
        per_core.append({"pk": pk})

    return per_core, {"n_real_cols": C, "c2": C2, "n2g": n2g}, chunk_tbl, C


# ---------------- device kernel ----------------
def build_kernel(C, chunk_tbl, num_devices=N_CORES, mm_bf16=True, n_real_cols=None,
                 c2=None, n2g=None):
    f = mybir.dt.float32
    h = mybir.dt.float16
    bf = mybir.dt.bfloat16
    if n_real_cols is None:
        n_real_cols = C
    if n2g is None:
        n2g = [GRP] * ((C + GRP - 1) // GRP)
    nc = bacc.Bacc("TRN2", target_bir_lowering=False, debug=False,
                   enable_asserts=False, num_devices=num_devices)
    u16 = mybir.dt.uint16
    d_pk = nc.dram_tensor("pk", [CHUNK, C, 8], u16, kind="ExternalInput")
    d_out = [nc.dram_tensor(f"out{p}", [CHUNK, PLANES * N_PIX_LO], f,
                            kind="ExternalOutput") for p in range(2)]

    with tile.TileContext(nc) as tc, ExitStack() as ctx:
        pool = ctx.enter_context(tc.tile_pool(name="sbuf", bufs=1))
        aypool = ctx.enter_context(tc.tile_pool(name="ay", bufs=6))
        axpool = ctx.enter_context(tc.tile_pool(name="ax", bufs=6))
        ppool = ctx.enter_context(tc.tile_pool(name="psum", bufs=1, space="PSUM"))

        tpk = pool.tile([CHUNK, C, 8], u16, tag="tpk")

        zl = pool.tile([CHUNK, CHUNK], bf, tag="zl")
        zr = pool.tile([CHUNK, 512], bf, tag="zr")
        nc.gpsimd.memset(zl[:], 0.0)
        nc.gpsimd.memset(zr[:], 0.0)

        # two psum images (y-block parity); 8 plane strips + 1 dumpster each
        imgs = [ppool.tile([CHUNK, PLANES + 1, N_PIX_LO], f, tag=f"img{p}",
                           space="PSUM", name=f"img{p}") for p in range(2)]
        for img in imgs:
            nc.tensor.matmul(out=img[:, 0:4, :], lhsT=zl[:], rhs=zr[:],
                             start=True, stop=False)
            nc.tensor.matmul(out=img[:, 4:8, :], lhsT=zl[:], rhs=zr[:],
                             start=True, stop=False)
            nc.tensor.matmul(out=img[:, 8:9, :], lhsT=zl[:], rhs=zr[:, 0:128],
                             start=True, stop=False)

        def vw(nm, asl, an, w):
            off = {"gx": (0, 2, f), "gy": (2, 4, f), "fw": (4, 5, h),
                   "tv0": (5, 6, h), "tv1": (6, 7, h)}[nm]
            v = tpk[:, asl, off[0]:off[1]].bitcast(off[2])
            return v.to_broadcast([CHUNK, an, w])

        glist = _groups(C)
        last_par_group = {0: -1, 1: -1}
        for g, (g0, gn) in enumerate(glist):
            pars = set(int(chunk_tbl[c, 1]) & 1
                       for c in range(g0, min(g0 + gn, n_real_cols)))
            for p in pars:
                last_par_group[p] = g

        def close_img(p):
            img = imgs[p]
            nc.tensor.matmul(out=img[:, 0:4, :], lhsT=zl[:], rhs=zr[:],
                             start=False, stop=True)
            nc.tensor.matmul(out=img[:, 4:8, :], lhsT=zl[:], rhs=zr[:],
                             start=False, stop=True)
            nc.tensor.matmul(out=img[:, 8:9, :], lhsT=zl[:], rhs=zr[:, 0:128],
                             start=False, stop=True)
            ot = pool.tile([CHUNK, PLANES * N_PIX_LO], f, tag=f"ot{p}",
                           name=f"ot{p}")
            nc.scalar.copy(out=ot[:, 0:512], in_=img[:, 0:4, :])
            nc.sync.dma_start(out=d_out[p].ap()[:, 0:512], in_=ot[:, 0:512])
            nc.scalar.copy(out=ot[:, 512:1024], in_=img[:, 4:8, :])
            nc.sync.dma_start(out=d_out[p].ap()[:, 512:1024],
                              in_=ot[:, 512:1024])

        for g, (g0, gn) in enumerate(glist):
            sl = slice(g0, g0 + gn)
            nc.sync.dma_start(out=tpk[:, sl, :], in_=d_pk.ap()[:, sl, :])
            k2 = int(n2g[g])                        # 2-slot cols in this group

            # x slots first, then per ay segment: trapezoid + its matmuls
            axp = axpool.tile([CHUNK, GRP, 2, WX], bf, tag="axp")
            for a0 in range(0, gn, AXSEG):
                an = min(AXSEG, gn - a0)
                asl = slice(g0 + a0, g0 + a0 + an)
                nc.vector._custom_dve(
                    TRAP_OP, out=axp[:, a0:a0 + an, 0, :],
                    in0=vw("tv0", asl, an, WX),
                    in1=vw("gx", asl, an, WX),
                    s1=4.0)
            for a0 in range(0, k2, AXSEG):
                an = min(AXSEG, k2 - a0)
                asl = slice(g0 + a0, g0 + a0 + an)
                nc.vector._custom_dve(
                    TRAP_OP, out=axp[:, a0:a0 + an, 1, :],
                    in0=vw("tv1", asl, an, WX),
                    in1=vw("gx", asl, an, WX),
                    s1=4.0)

            ay = aypool.tile([CHUNK, GRP, WY], bf, tag="ay")
            for a0 in range(0, gn, AYSEG):
                an = min(AYSEG, gn - a0)
                asl = slice(g0 + a0, g0 + a0 + an)
                nc.vector._custom_dve(
                    TRAP_OP, out=ay[:, a0:a0 + an, :],
                    in0=vw("fw", asl, an, WY),
                    in1=vw("gy", asl, an, WY),
                    s1=4.0)
                for c in range(g0 + a0, min(g0 + a0 + an, n_real_cols)):
                    plane, yb, xblk = (int(chunk_tbl[c, 0]),
                                       int(chunk_tbl[c, 1]),
                                       int(chunk_tbl[c, 2]))
                    par = yb & 1
                    r = yb >> 1
                    wx = min(WX, N_PIX_LO - xblk * 8)
                    nsl = 2 if (c - g0) < k2 else 1
                    nc.tensor.matmul(
                        out=imgs[par][32 * r:32 * r + WY, plane:plane + nsl,
                                      xblk * 8:xblk * 8 + wx],
                        lhsT=ay[:, c - g0, :],
                        rhs=axp[:, c - g0, 0:nsl, 0:wx],
                        start=False, stop=False,
                        tile_position=(0, 32 * r))
            for p in (0, 1):
                if last_par_group[p] == g:
                    close_img(p)

        for p in (0, 1):
            if last_par_group[p] == -1:             # parity never touched
                close_img(p)

    nc.compile()
    return nc


def assemble(results):
    cube = np.empty((NV_LO, N_PIX_LO, N_PIX_LO), np.float32)
    for k in range(N_CORES):
        # img_par partition 32r+m (m<16) holds y cell 32r + 16*par + m
        for par in range(2):
            res = results[k][f"out{par}"].reshape(4, 32, PLANES, N_PIX_LO)
            # res[r, m] valid only for m < 16
            sub = res[:, 0:16]                      # [4, 16, PLANES, 128]
            for r in range(4):
                y0 = 32 * r + 16 * par
                cube[k * PLANES:(k + 1) * PLANES, y0:y0 + 16] = (
                    sub[r].transpose(1, 0, 2))
    return cube


# ---------------- entry point ----------------
def kernel(ra, dec, vel, flux):
    per_core, consts, chunk_tbl, C = route_points(ra, dec, vel, flux)
    if C == 0:  # no valid points at all
        return np.zeros((NV_LO, N_PIX_LO, N_PIX_LO), np.float32)
    _log(f"C={C} columns ({C * CHUNK} entry slots)")
    nc = build_kernel(C, chunk_tbl, n_real_cols=consts["n_real_cols"],
                      c2=consts["c2"], n2g=consts["n2g"])
    in_maps = [dict(per_core[k]) for k in range(N_CORES)]
    res = run_bass_kernel_spmd(nc, in_maps, core_ids=list(range(N_CORES)))
    return assemble(res.results)
